# revision 43
# baseline (speedup 1.0000x reference)
"""MoE (8 experts, top-2, SwiGLU FFN) Trainium2 kernel.

Sharding: data-parallel over tokens. Each of the 8 cores gets T/8 = 512
tokens and computes the full MoE for them: router (fp32 matmul + softmax +
top-2 via max/second-max thresholding) and all 8 experts' FFNs (fp32r
matmuls), accumulating cw-weighted expert outputs on-chip. Host only
reshapes/transposes inputs and concatenates the 8 output slices.

Schedule notes (cost-model driven):
 - A few discarded f32r matmuls warm the PE (HAM ramp) before the fp32
   router so the router runs at full clock (853ns vs 2429ns per matmul).
 - DMA issue order: rwt, x (per-d-tile chunks), b2, b1, then per-expert
   w1, (b3,) w3, w2 — so the first matmuls of each stage start as soon as
   their first operand lands.
 - The router->combine-weight chain (transpose, softmax, top-2) runs
   entirely on DVE/ACT (32x32 stream transposes + 4 tiny partition-shift
   DMAs on the gpsimd queue), so the PE stream never interleaves with it.
 - Output is written per (t-tile, d-chunk) to a DRAM-contiguous buffer;
   the host undoes the tiling permutation for free.

Layouts inside a core (partition dim first):
  xT      [128(d%128), 8(d//128), 512(t)]    moving operand of mm1/router
  w1T/w3T [128(d%128), 8(d//128), 512(h)]    stationary tiles [d,h] for mm1
  h/u     PSUM [128(h%128), 512(t)]          per h-tile, accum over d-tiles
  gu      [128(h%128), 4(h//128), 512(t)]    stationary tiles [h,t] for mm2
  w2T     [128(h%128), 4(h//128), 1024(d)]   moving operand of mm2
  y       PSUM [128(t%128), 512(d-chunk)]    accum over h-tiles
  out_acc [128(t%128), 4(t//128), 1024(d)]   sum_e cw_e * (y_e + b2_e)
"""

import numpy as np

import concourse.bass as bass
import concourse.bacc as bacc
import concourse.mybir as mybir
import concourse.tile as tile

D, H, E, T = 1024, 512, 8, 4096
NCORES = 8
TLOC = T // NCORES          # 512 tokens per core
DT = D // 128               # 8 d-tiles
HT = H // 128               # 4 h-tiles
TT = TLOC // 128            # 4 t-tiles
DC = D // 512               # 2 d-chunks for mm2 moving operand
N_WARM = 5                  # discarded matmuls to ramp the PE clock
F32 = mybir.dt.float32
F32R = mybir.dt.float32r
AX = mybir.AluOpType


def _bc(ap, n):
    """Append a step-0 (broadcast) innermost free dim of size n."""
    return ap.broadcast_to([*ap.shape, n])


def build_nc(loop_n=1):
    nc = bacc.Bacc("TRN2", target_bir_lowering=False, debug=False,
                   num_devices=NCORES)

    xtf = nc.dram_tensor("xtf", [DT, 128, TLOC], F32, kind="ExternalInput")
    rwt = nc.dram_tensor("rwt", [DT, 128, E], F32, kind="ExternalInput")
    w1t = nc.dram_tensor("w1t", [E, DT, 128, H], F32R, kind="ExternalInput")
    w3t = nc.dram_tensor("w3t", [E, DT, 128, H], F32R, kind="ExternalInput")
    w2t = nc.dram_tensor("w2t", [E, HT, 128, D], F32R, kind="ExternalInput")
    b1t = nc.dram_tensor("b1t", [E, HT, 128], F32, kind="ExternalInput")
    b3t = nc.dram_tensor("b3t", [E, HT, 128], F32, kind="ExternalInput")
    b2 = nc.dram_tensor("b2", [E, D], F32R, kind="ExternalInput")
    out = nc.dram_tensor("out", [TT, DC, 128, 512], F32, kind="ExternalOutput")

    import contextlib
    with tile.TileContext(nc) as tc:
        # loop_n > 1 replays the identical body via a hardware loop — used
        # only by test.py's loop-differencing timer (kernel() uses loop_n=1).
        # The PE body is ~650 instructions (> one IRAM block), so without a
        # branch hint every back-edge stalls ~3-4us on the IRAM refetch.
        loop_cm = (tc.For_i(0, loop_n, 1,
                            hint_engines=(mybir.EngineType.PE,))
                   if loop_n > 1 else contextlib.nullcontext())
        with (
            tc.tile_pool(name="singles", bufs=1) as singles,
            tc.tile_pool(name="wpool", bufs=2) as wpool,
            tc.tile_pool(name="gpool", bufs=2) as gpool,
            tc.tile_pool(name="pmm", bufs=6, space="PSUM") as pmm,
            tc.tile_pool(name="psmall", bufs=2, space="PSUM") as psmall,
            loop_cm,
        ):
            # ---- one-time loads (order = DMA queue order) ------------------
            rwt_sb = singles.tile([128, DT, E], F32)
            nc.sync.dma_start(out=rwt_sb, in_=rwt.ap().rearrange("a p e -> p a e"))
            # x lands once as fp32 (router needs true fp32); the f32r FFN
            # copy is made on-chip by the otherwise-idle DVE (saves 2MB HBM)
            xtf_sb = singles.tile([128, DT, TLOC], F32)
            xtf_r = xtf.ap().rearrange("a p t -> p a t")
            for dt in range(DT):
                nc.sync.dma_start(out=xtf_sb[:, dt, :], in_=xtf_r[:, dt, :])
            xt_sb = singles.tile([128, DT, TLOC], F32R)
            for dt in range(DT):
                nc.vector.tensor_copy(xt_sb[:, dt, :], xtf_sb[:, dt, :])
            b2_sb = singles.tile([E, D], F32R)
            nc.sync.dma_start(out=b2_sb, in_=b2.ap())
            b1_sb = singles.tile([128, E, HT], F32)
            nc.sync.dma_start(out=b1_sb, in_=b1t.ap().rearrange("e h p -> p e h"))
            dume = singles.tile([1, 1], F32)
            nc.scalar.activation(dume, rwt_sb[0:1, 0, 0:1],
                                 mybir.ActivationFunctionType.Exp)

            # ---- PE warm-up: discarded f32r matmuls ------------------------
            p_warm = psmall.tile([128, TLOC], F32, tag="small")
            for _ in range(N_WARM):
                nc.tensor.matmul(p_warm, xt_sb[:, 0, 0:128], xt_sb[:, 0, :],
                                 start=True, stop=True)

            # ---- router: logitsT[e, t] = (router_w @ x.T) ------------------
            # full fp32 so top-2 selection matches the fp32 reference
            p_lg = psmall.tile([32, TLOC], F32, tag="small")
            nc.vector.memset(p_lg, 0.0)
            for dt in range(DT):
                nc.tensor.matmul(p_lg[0:E, :], rwt_sb[:, dt, :],
                                 xtf_sb[:, dt, :],
                                 start=(dt == 0), stop=(dt == DT - 1))
            # transpose logitsT straight out of PSUM on the DVE (32x32 block
            # transpose) so no PE op or copy sits in the router->cw chain
            lgT32 = singles.tile([32, 16, 32], F32)
            nc.vector.transpose(lgT32.rearrange("p a e -> p (a e)"), p_lg)
            # token t = 32*b + i lives at [i, b, e] for e < 8

            # softmax over e (no max-subtraction needed: logits ~ N(0,1));
            # scores32 doubles as the dense combine-weight tile (cols 8+ stay 0)
            sl = lgT32[:, :, 0:E]
            scores32 = singles.tile([32, 16, 32], F32)
            nc.vector.memset(scores32, 0.0)
            sc = scores32[:, :, 0:E]
            nc.scalar.activation(sc, sl, mybir.ActivationFunctionType.Exp)
            ssum = singles.tile([32, 16], F32)
            nc.vector.reduce_sum(ssum, sc, axis=mybir.AxisListType.X)
            rsum = singles.tile([32, 16], F32)
            nc.vector.reciprocal(rsum, ssum)
            nc.vector.tensor_tensor(sc, sc, _bc(rsum, E), op=AX.mult)

            # top-2: cw = score * (score >= second_max)
            m1 = singles.tile([32, 16], F32)
            nc.vector.reduce_max(m1, sc, axis=mybir.AxisListType.X)
            tmp32 = singles.tile([32, 16, E], F32)
            nc.vector.tensor_tensor(tmp32, sc, _bc(m1, E), op=AX.is_equal)
            nc.vector.scalar_tensor_tensor(tmp32, tmp32, -1e30, sc,
                                           op0=AX.mult, op1=AX.add)
            m2 = singles.tile([32, 16], F32)
            nc.vector.reduce_max(m2, tmp32, axis=mybir.AxisListType.X)
            nc.vector.tensor_tensor(tmp32, sc, _bc(m2, E), op=AX.is_ge)
            nc.vector.tensor_tensor(sc, sc, tmp32, op=AX.mult)

            # cwT[e, t] via a second DVE block transpose (rows 8+ are junk)
            cwTp = singles.tile([32, 16, 32], F32)
            nc.vector.transpose(cwTp.rearrange("p a e -> p (a e)"),
                                scores32.rearrange("p a e -> p (a e)"))
            cwT = singles.tile([E, 16, 32], F32R)
            nc.vector.tensor_copy(cwT, cwTp[0:E, :, :])

            # cw in [t%128, tt, e] layout for the y-combine scalars:
            # 4 tiny partition-shift DMAs (gpsimd queue; sync queue carries
            # the big weight streams and must not head-of-line block on cw)
            cw128 = singles.tile([128, TT, E], F32)
            cw_v = scores32.rearrange("p (t q) e -> p t q e", q=4)
            for q in range(4):
                nc.gpsimd.dma_start(out=cw128[32 * q:32 * (q + 1), :, :],
                                    in_=cw_v[:, :, q, 0:E])

            def emit_expert_hu(e, w1_sb, w3_sb, w2_sb):
                g_sb = gpool.tile([128, HT, TLOC], F32, tag="g")
                hb_sb = gpool.tile([128, HT, TLOC], F32, tag="hb")
                gu_sb = gpool.tile([128, HT, TLOC], F32R, tag="gu")
                for ht in range(HT):
                    hs = slice(ht * 128, (ht + 1) * 128)
                    p_h = pmm.tile([128, TLOC], F32, tag="mm")
                    for dt in range(DT):
                        nc.tensor.matmul(p_h, w1_sb[:, dt, hs], xt_sb[:, dt, :],
                                         start=(dt == 0), stop=(dt == DT - 1))
                    # silu(h+b1)*(u+b3) = (h+b1)*sigmoid(h+b1)*(u+b3)
                    nc.scalar.activation(g_sb[:, ht, :], p_h,
                                         mybir.ActivationFunctionType.Sigmoid,
                                         bias=b1_sb[:, e, ht:ht + 1], scale=1.0)
                    nc.vector.tensor_scalar_add(hb_sb[:, ht, :], p_h,
                                                b1_sb[:, e, ht:ht + 1])
                for ht in range(HT):
                    hs = slice(ht * 128, (ht + 1) * 128)
                    p_u = pmm.tile([128, TLOC], F32, tag="mm")
                    for dt in range(DT):
                        last_u = nc.tensor.matmul(p_u, w3_sb[:, dt, hs],
                                                  xt_sb[:, dt, :],
                                                  start=(dt == 0),
                                                  stop=(dt == DT - 1))
                    nc.vector.scalar_tensor_tensor(gu_sb[:, ht, :], p_u,
                                                   b3_sb[:, e, ht:ht + 1],
                                                   g_sb[:, ht, :],
                                                   op0=AX.add, op1=AX.mult)
                    nc.vector.tensor_mul(gu_sb[:, ht, :], gu_sb[:, ht, :],
                                         hb_sb[:, ht, :])
                return gu_sb, last_u

            def emit_expert_y(e, gu_sb, w2_sb):
                # y[t, d] = gu.T @ w2T ; out_acc += cw_e * y
                for tt in range(TT):
                    ts_ = slice(tt * 128, (tt + 1) * 128)
                    for dc in range(DC):
                        ds_ = slice(dc * 512, (dc + 1) * 512)
                        p_y = pmm.tile([128, 512], F32, tag="mm")
                        for ht in range(HT):
                            nc.tensor.matmul(p_y, gu_sb[:, ht, ts_],
                                             w2_sb[:, ht, ds_],
                                             start=(ht == 0), stop=(ht == HT - 1))
                        nc.vector.scalar_tensor_tensor(
                            out_acc[:, tt, ds_], p_y, cw128[:, tt, e:e + 1],
                            out_acc[:, tt, ds_], op0=AX.mult, op1=AX.add)

            def emit_expert_dmas(e):
                w1_sb = wpool.tile([128, DT, H], F32R, tag="w1")
                w1_src = w1t.ap()[e].rearrange("a p h -> p a h")
                if e == 0:
                    # first-use latency: land h-tile 0 first so mm1 starts
                    # right after the router instead of waiting for all 4MB
                    for ht in range(HT):
                        hs = slice(ht * 128, (ht + 1) * 128)
                        nc.sync.dma_start(out=w1_sb[:, :, hs],
                                          in_=w1_src[:, :, hs])
                else:
                    nc.sync.dma_start(out=w1_sb, in_=w1_src)
                if e == 0:
                    nc.sync.dma_start(out=b3_sb,
                                      in_=b3t.ap().rearrange("e h p -> p e h"))
                w3_sb = wpool.tile([128, DT, H], F32R, tag="w3")
                nc.sync.dma_start(out=w3_sb,
                                  in_=w3t.ap()[e].rearrange("a p h -> p a h"))
                w2_sb = wpool.tile([128, HT, D], F32R, tag="w2")
                nc.sync.dma_start(out=w2_sb,
                                  in_=w2t.ap()[e].rearrange("a p d -> p a d"))
                return w1_sb, w3_sb, w2_sb

            # out_acc = cw @ b2 (the bias part of the combine)
            b3_sb = singles.tile([128, E, HT], F32)
            out_acc = singles.tile([128, TT, D], F32)
            for tt in range(TT):
                for dc in range(DC):
                    p_b = pmm.tile([128, 512], F32, tag="mm")
                    nc.tensor.matmul(p_b, cwT[:, 4 * tt:4 * (tt + 1), :],
                                     b2_sb[:, dc * 512:(dc + 1) * 512])
                    nc.vector.tensor_copy(out_acc[:, tt, dc * 512:(dc + 1) * 512],
                                          p_b)

            # software pipeline one expert ahead: emit mm1(e+1) before
            # mm2(e) so mm2 never waits on the DVE-produced gu tile
            # (gpool/wpool bufs=2 hold exactly two experts in flight)
            pend = None
            for e in range(E):
                w1_sb, w3_sb, w2_sb = emit_expert_dmas(e)
                gu_sb, _ = emit_expert_hu(e, w1_sb, w3_sb, w2_sb)
                if pend is not None:
                    emit_expert_y(*pend)
                pend = (e, gu_sb, w2_sb)
            emit_expert_y(*pend)

            # ---- store (chunked + DRAM-contiguous; host re-lays-out) -------
            out_r = out.ap().rearrange("a b p d -> p a b d")
            for tt in range(TT):
                for dc in range(DC):
                    nc.sync.dma_start(out=out_r[:, tt, dc, :],
                                      in_=out_acc[:, tt,
                                                  dc * 512:(dc + 1) * 512])

    nc.compile()
    return nc


_NC_CACHE = None


def _get_nc():
    global _NC_CACHE
    if _NC_CACHE is None:
        _NC_CACHE = build_nc()
    return _NC_CACHE


def make_in_maps(x, router_w, w1, b1, w3, b3, w2, b2):
    xt_full = np.ascontiguousarray(x.reshape(T, D))
    shared = {
        "rwt": np.ascontiguousarray(router_w.T).reshape(DT, 128, E),
        "w1t": np.ascontiguousarray(w1.transpose(0, 2, 1)).reshape(E, DT, 128, H),
        "w3t": np.ascontiguousarray(w3.transpose(0, 2, 1)).reshape(E, DT, 128, H),
        "w2t": np.ascontiguousarray(w2.transpose(0, 2, 1)).reshape(E, HT, 128, D),
        "b1t": np.ascontiguousarray(b1).reshape(E, HT, 128),
        "b3t": np.ascontiguousarray(b3).reshape(E, HT, 128),
        "b2": np.ascontiguousarray(b2),
    }
    shared = {k: v.astype(np.float32, copy=False) for k, v in shared.items()}
    in_maps = []
    for c in range(NCORES):
        xc = xt_full[c * TLOC:(c + 1) * TLOC]
        xtc = np.ascontiguousarray(xc.T).reshape(DT, 128, TLOC)
        in_maps.append(dict(shared, xtf=xtc))
    return in_maps


def assemble_output(per_core):
    """per_core: list (one per core) of dicts with the 'out' array."""
    outs = [np.asarray(per_core[c]["out"]).transpose(0, 2, 1, 3
                                                     ).reshape(TLOC, D)
            for c in range(NCORES)]
    return np.concatenate(outs, axis=0).reshape(4, 1024, D)


def kernel(x, router_w, w1, b1, w3, b3, w2, b2):
    from concourse.bass_utils import run_bass_kernel_spmd

    nc = _get_nc()
    in_maps = make_in_maps(np.asarray(x, dtype=np.float32),
                           np.asarray(router_w, dtype=np.float32),
                           np.asarray(w1, dtype=np.float32),
                           np.asarray(b1, dtype=np.float32),
                           np.asarray(w3, dtype=np.float32),
                           np.asarray(b3, dtype=np.float32),
                           np.asarray(w2, dtype=np.float32),
                           np.asarray(b2, dtype=np.float32))
    res = run_bass_kernel_spmd(nc, in_maps, core_ids=list(range(NCORES)))
    return assemble_output(res.results)

